# revision 20
# baseline (speedup 1.0000x reference)
"""Trainium2 Bass kernel for nn_MiddleOut (gnn_message_passing).

Math (reference), with P = #peers and W = [Wm | Wp | wm] along the in dim:
    out = (1/P) * [ s1*(my@Wm.T + bias) + z@Wp.T + s2*wm ]
    s1[b] = sum_p m[b,p];  s2[b] = sum_p m[b,p]^2
    z[b,l] = sum_p m[b,p] * peer[b,p,l]

Sharding: pure data parallel over batch across 8 cores (2048 rows/core,
16 tiles of 128).

On-device strategy per tile:
  - peers are cast to fp8 e3m4 on host (4 mantissa bits; |peer|max ~5.4 well
    under the 15.5 format max), halving the dominant HBM stream vs bf16;
    measured end-to-end absmax-rel ~4.3e-3 vs the 2e-2 budget.
  - the weighted peer-reduction z runs as FOUR CONCURRENT column-tiled
    matmul chains (tile_position col groups j=0..3, 32 output partitions
    each): stationary = a [128,32] metric band slab, moving = the [128,256]
    peer slab. Column tiling streams 4 moving operands through the PE at
    once, cutting the z wall-time ~4x vs one 128-wide chain.
    Batch mapping: b_loc = 32*j + 8*b4 + k, slab s = 4*k + j, band column
    c = k + 8*b4 holds m[b_loc, p] at rows (b4,p).
  - projection avoids transposing [u|z]: my is host-transposed (myT) so
    A = my@Wm'/P comes from 2 direct chains; only z (2 chunks, bf16) and
    s12 (tiny) are PE-transposed, then 3 chains give
    psum_o = z@Wp' + s1*bias' + s2*wm' (all 1/P pre-folded on host).
  - final combine on DVE: out = A_sb * s1 + psum_o via scalar_tensor_tensor
    (s1, s2 are f32 DVE reductions of the bf16 metrics).
  - DMA issue cost (~0.6us of sequencer time per dma_start) is kept off the
    busy engines: x is one 1MB issue on sync; the merged meta|myT load and
    the out store issue from the otherwise-idle GpSimd (SWDGE).
"""

import ml_dtypes
import numpy as np

import concourse.bass as bass
import concourse.mybir as mybir
import concourse.tile as tile
from concourse import bacc
from concourse.bass_utils import run_bass_kernel_spmd

F32 = mybir.dt.float32
BF = mybir.dt.bfloat16
E3 = mybir.dt.float8e3

B, P, L, R = 16384, 32, 256, 256
N_CORES = 8
BC = B // N_CORES          # 2048 batches per core
TILE_B = 128               # batches per SBUF tile
NT = BC // TILE_B          # 16 tiles
NJ = 4                     # column-tile groups (32 out partitions each)
NK = 8                     # chained matmuls per column group
NS = NJ * NK               # 32 peer slabs per tile
MW = 2 * P + 2 * TILE_B    # meta row: [m_t | mb | myT chunk0 | myT chunk1]

_cache = {}


def build_bass(nt=NT, num_devices=N_CORES):
    bc = nt * TILE_B
    nc = bacc.Bacc(
        "TRN2", target_bir_lowering=False, debug=False, num_devices=num_devices
    )

    x_d = nc.dram_tensor("x", [nt, TILE_B, NS, L], E3, kind="ExternalInput")
    meta_d = nc.dram_tensor("meta", [nt, TILE_B, MW], BF, kind="ExternalInput")
    # wbf packs [WmT' | WpT' | identity | w45'] (bf16); identf is f32 identity
    wbf_d = nc.dram_tensor("wbf", [TILE_B, 4 * L + TILE_B + R], BF, kind="ExternalInput")
    out_d = nc.dram_tensor("out", [bc, R], BF, kind="ExternalOutput")

    with TileCtx(nc) as (tc, ctx):
        singles = ctx.enter_context(tc.tile_pool(name="singles", bufs=1))
        xp = ctx.enter_context(tc.tile_pool(name="xp", bufs=6))
        small = ctx.enter_context(tc.tile_pool(name="small", bufs=4))
        xtp = ctx.enter_context(tc.tile_pool(name="xtp", bufs=3))
        psz = ctx.enter_context(tc.tile_pool(name="psz", bufs=2, space="PSUM"))
        pst = ctx.enter_context(tc.tile_pool(name="pst", bufs=2, space="PSUM"))
        pso = ctx.enter_context(tc.tile_pool(name="pso", bufs=2, space="PSUM"))
        psa = ctx.enter_context(tc.tile_pool(name="psa", bufs=1, space="PSUM"))

        w_sb = singles.tile([TILE_B, 4 * L + TILE_B + R], BF)
        wm_sb = w_sb[:, 0:2 * L]                     # [128, 2*256] WmT'/P
        wp_sb = w_sb[:, 2 * L:4 * L]                 # [128, 2*256] WpT'/P
        identb = w_sb[:, 4 * L:4 * L + TILE_B]       # [128, 128] bf16 identity
        w45 = w_sb[:, 4 * L + TILE_B:]               # rows 0:2 = [bias'; wm']/P

        # Ping-pong block-diagonal stationaries for the weighted peer-reduce.
        # Slab s = 4k+j is [128, 32]: column k+8*b4 holds m[32j+8b4+k, p] at
        # rows (b4,p); zeros written once, the band rewritten every tile.
        s_tiles = []
        for i in range(3):
            s_i = singles.tile([TILE_B, NS, P], E3, tag=f"s{i}")
            nc.vector.memset(s_i.bitcast(F32), 0.0)
            s_tiles.append(s_i)

        # Two-stage software pipeline over the PE stream so its queue never
        # head-of-line blocks on ACT evacuations: per iteration i the PE gets
        # [chains(i-2), z(i), A(i), transposes(i-1)] — every group is ready
        # when it reaches the queue head, keeping the PE dense (HAM warm).
        # The metric band for tile i+1 is DVE-written one stage early so the
        # next iteration's z never waits on it.
        st = {}  # per-tile live tiles

        def load_meta(t):
            meta = small.tile([TILE_B, MW], BF, tag="meta")
            nc.sync.dma_start(out=meta, in_=meta_d[t])
            st[t] = {"meta": meta}

        def band_s12(t):
            meta = st[t]["meta"]
            # band: element (32*b4+p, slab 4k+j, col k+8*b4): off = 129k+32j+8b4
            s_all = s_tiles[t % 3]
            for b4 in range(4):
                view = s_all[b4 * P:(b4 + 1) * P, :, :]
                out_ap = bass.AP(
                    tensor=view.tensor, offset=view.offset + 8 * b4,
                    ap=[view.ap[0], [32, NJ], [129, NK]],
                )
                mtv = meta[b4 * P:(b4 + 1) * P, 0:P]
                in_ap = bass.AP(
                    tensor=mtv.tensor, offset=mtv.offset,
                    ap=[mtv.ap[0], [1, NJ], [4, NK]],
                )
                nc.vector.tensor_copy(out=out_ap, in_=in_ap)
            mb = meta[:, P:2 * P]
            m2 = small.tile([TILE_B, P], BF, tag="m2")
            nc.vector.tensor_mul(m2, mb, mb)
            s12 = small.tile([TILE_B, 2], F32, tag="s12")  # [s1 | s2]
            nc.vector.tensor_reduce(
                out=s12[:, 0:1], in_=mb, axis=mybir.AxisListType.X,
                op=mybir.AluOpType.add,
            )
            nc.vector.tensor_reduce(
                out=s12[:, 1:2], in_=m2, axis=mybir.AxisListType.X,
                op=mybir.AluOpType.add,
            )
            st[t]["s12"] = s12

        def z_and_a(t):
            x_t = xp.tile([TILE_B, NS, L], E3, tag="x_t")
            # chunk the first loads so tile-0/1/2 matmuls start on partial
            # data (each chunk of NS//8 slabs = one k-group of 4)
            nch = 8 if t == 0 else (4 if t == 1 else (2 if t == 2 else 1))
            step = NS // nch
            for q in range(nch):
                nc.sync.dma_start(
                    out=x_t[:, q * step:(q + 1) * step, :],
                    in_=x_d[t, :, q * step:(q + 1) * step, :],
                )
            st[t]["x_t"] = x_t

        def a_mms(t):
            meta = st[t]["meta"]
            psum_a = psa.tile([TILE_B, R], F32, tag="psum_a")
            nc.tensor.matmul(
                out=psum_a, lhsT=meta[:, 2 * P:2 * P + TILE_B],
                rhs=wm_sb[:, 0:R], start=True, stop=False,
            )
            nc.tensor.matmul(
                out=psum_a, lhsT=meta[:, 2 * P + TILE_B:MW],
                rhs=wm_sb[:, R:2 * R], start=False, stop=True,
            )
            a_sb = small.tile([TILE_B, R], BF, tag="a_sb")
            nc.scalar.copy(out=a_sb, in_=psum_a)
            st[t]["a_sb"] = a_sb

        def z_mms(t):
            x_t = st[t]["x_t"]
            s_all = s_tiles[t % 3]
            psum_z = psz.tile([TILE_B, L], F32, tag="psum_z")
            for k in range(NK):
                for j in range(NJ):
                    s = 4 * k + j
                    nc.tensor.matmul(
                        out=psum_z[32 * j:32 * (j + 1), :],
                        lhsT=s_all[:, s, :],
                        rhs=x_t[:, s, :],
                        start=(k == 0),
                        stop=(k == NK - 1),
                        tile_position=(0, 32 * j),
                        skip_group_check=True,
                    )
            z_sb = small.tile([TILE_B, L], BF, tag="z_sb")
            nc.scalar.copy(out=z_sb, in_=psum_z)
            st[t]["z_sb"] = z_sb

        def transposes(t):
            z_sb, s12 = st[t]["z_sb"], st[t]["s12"]
            zts = []
            for c in range(2):
                ptz = pst.tile([TILE_B, TILE_B], BF, tag="ptz")
                nc.tensor.transpose(
                    out=ptz, in_=z_sb[:, c * TILE_B:(c + 1) * TILE_B],
                    identity=identb,
                )
                zt = xtp.tile([TILE_B, TILE_B], BF, tag=f"zt{c}")
                nc.scalar.copy(out=zt, in_=ptz)
                zts.append(zt)
            s12b = small.tile([TILE_B, 2], BF, tag="s12b")
            nc.vector.tensor_copy(out=s12b, in_=s12)
            pts = pst.tile([TILE_B, TILE_B], BF, tag="pts", bufs=1)
            nc.tensor.transpose(out=pts[0:2, :], in_=s12b, identity=identb)
            s12t = xtp.tile([2, TILE_B], BF, tag="s12t")
            nc.scalar.copy(out=s12t, in_=pts[0:2, :])
            st[t]["zts"] = zts
            st[t]["s12t"] = s12t

        def chains_out(t):
            zts, s12t, s12, a_sb = (
                st[t]["zts"], st[t]["s12t"], st[t]["s12"], st[t]["a_sb"]
            )
            psum_o = pso.tile([TILE_B, R], F32, tag="psum_o")
            nc.tensor.matmul(
                out=psum_o, lhsT=zts[0], rhs=wp_sb[:, 0:R],
                start=True, stop=False,
            )
            nc.tensor.matmul(
                out=psum_o, lhsT=zts[1], rhs=wp_sb[:, R:2 * R],
                start=False, stop=False,
            )
            nc.tensor.matmul(
                out=psum_o, lhsT=s12t, rhs=w45[0:2, :],
                start=False, stop=True,
            )
            out_sb = small.tile([TILE_B, R], BF, tag="out_sb")
            nc.vector.scalar_tensor_tensor(
                out=out_sb, in0=a_sb, scalar=s12[:, 0:1], in1=psum_o,
                op0=mybir.AluOpType.mult, op1=mybir.AluOpType.add,
            )
            nc.gpsimd.dma_start(
                out=out_d[t * TILE_B:(t + 1) * TILE_B, :], in_=out_sb
            )
            del st[t]

        load_meta(0)
        nc.sync.dma_start(out=w_sb, in_=wbf_d[:, :])
        band_s12(0)
        for i in range(nt + 2):
            if 2 <= i:
                chains_out(i - 2)
            if i < nt:
                if i + 1 < nt:
                    load_meta(i + 1)
                z_and_a(i)      # x DMA issue only
                a_mms(i)
            if 1 <= i <= nt:
                transposes(i - 1)
            if i < nt:
                z_mms(i)
                if i + 1 < nt:
                    band_s12(i + 1)

    nc.compile()
    return nc


class TileCtx:
    """with TileCtx(nc) as (tc, ctx): — TileContext plus an ExitStack."""

    def __init__(self, nc):
        from contextlib import ExitStack
        self.tc = tile.TileContext(nc)
        self.ctx = ExitStack()

    def __enter__(self):
        return self.tc.__enter__(), self.ctx.__enter__()

    def __exit__(self, *a):
        self.ctx.__exit__(*a)
        return self.tc.__exit__(*a)


def prep_inputs(my_latent, peer_latents, peer_metrics, W, b, nt=NT, n_cores=N_CORES):
    """Host-side shard + layout prep (dtype casts and permutes; 1/P folded
    into the weight pack)."""
    E3np = ml_dtypes.float8_e3m4
    BFnp = ml_dtypes.bfloat16
    bc = nt * TILE_B

    wbf = np.zeros((TILE_B, 4 * L + TILE_B + R), dtype=np.float32)
    wt = np.ascontiguousarray(W.T) / P                   # [513, 256] pre-scaled
    wbf[:, 0:R] = wt[0:TILE_B]
    wbf[:, R:2 * R] = wt[TILE_B:2 * TILE_B]
    wbf[:, 2 * L:2 * L + R] = wt[L:L + TILE_B]
    wbf[:, 2 * L + R:4 * L] = wt[L + TILE_B:L + 2 * TILE_B]
    wbf[:, 4 * L:4 * L + TILE_B] = np.eye(TILE_B, dtype=np.float32)
    wbf[0, 4 * L + TILE_B:] = b / P                      # bias'
    wbf[1, 4 * L + TILE_B:] = wt[2 * L]                  # wm'
    wbf = wbf.astype(BFnp)

    # batch scramble within a tile: b_loc = 32j + 8*b4 + k
    b4r = np.arange(4)[:, None, None]
    kr = np.arange(NK)[None, :, None]
    jr = np.arange(NJ)[None, None, :]
    bl_map = 32 * jr + 8 * b4r + kr                      # [4, 8, 4] (b4, k, j)

    in_maps = []
    for c in range(n_cores):
        sl = slice(c * bc, (c + 1) * bc)
        peer_c = peer_latents[sl].reshape(nt, TILE_B, P, L)
        m_c = peer_metrics[sl].reshape(nt, TILE_B, P)
        my_c = my_latent[sl].reshape(nt, TILE_B, L)

        # x[t, 32*b4+p, 4k+j, l] = peer[bl_map, p, l]
        xc = peer_c[:, bl_map, :, :]                     # [nt, 4, 8, 4, P, L]
        xc = np.ascontiguousarray(
            xc.transpose(0, 1, 4, 2, 3, 5)               # t, b4, p, k, j, l
        ).reshape(nt, TILE_B, NS, L).astype(E3np)

        # meta: [m_t | mb | myT]; m_t[t, 32*b4+p, 4k+j] = m[bl_map, p]
        meta = np.empty((nt, TILE_B, MW), dtype=np.float32)
        mt = m_c[:, bl_map, :]                           # [nt, 4, 8, 4, P]
        meta[:, :, 0:P] = mt.transpose(0, 1, 4, 2, 3).reshape(nt, TILE_B, P)
        meta[:, :, P:2 * P] = m_c
        # myT[t, lp, c2*128 + b] = my[b, c2*128+lp]
        meta[:, :, 2 * P:] = my_c.reshape(nt, TILE_B, 2, TILE_B).transpose(
            0, 3, 2, 1).reshape(nt, TILE_B, 2 * TILE_B)
        meta = meta.astype(BFnp)

        in_maps.append({
            "x": xc,
            "meta": meta,
            "wbf": wbf,
        })
    return in_maps


def run(my_latent, peer_latents, peer_metrics, W, b, trace=False, **kw):
    if "nc" not in _cache:
        _cache["nc"] = build_bass()
    nc = _cache["nc"]
    in_maps = prep_inputs(
        np.asarray(my_latent, dtype=np.float32),
        np.asarray(peer_latents, dtype=np.float32),
        np.asarray(peer_metrics, dtype=np.float32),
        np.asarray(W, dtype=np.float32),
        np.asarray(b, dtype=np.float32),
    )
    res = run_bass_kernel_spmd(
        nc, in_maps, core_ids=list(range(N_CORES)), trace=trace, **kw
    )
    out = np.concatenate([r["out"] for r in res.results], axis=0).astype(np.float32)
    return out, res


def kernel(my_latent, peer_latents, peer_metrics, W, b):
    out, _ = run(my_latent, peer_latents, peer_metrics, W, b)
    return out
